# revision 3
# baseline (speedup 1.0000x reference)
"""2-layer GCN on 8 Trainium2 cores via Bass/Tile (fp16, v3.3).

Aggregation via identity-packed gathers: within each (128-dest block x src
chunk) segment, edges are packed into rounds with at most one edge per dest;
the gather places the edge for dest p at partition p, so the aggregation
matmul uses a CONSTANT identity S (no per-group DVE one-hot). Pad slots fetch
a reserved all-zero table row (padding nodes are interleaved per quarter-shard
so every chunk contains zero rows). Leftover multiplicity>R edges go to
classic one-hot groups (~1 per segment, single-op is_equal on DVE).

Also: separable norm (dinv folded into tables + per-dest multiply),
self-loops as one identity matmul per 128-block from the SBUF-resident shard,
bucket-level gathers (512 dests x chunk -> 100 dma_gather calls/layer),
half-table AllGathers to pipeline collectives with compute.

Node placement: core c holds real nodes [c*12500,(c+1)*12500); its shard of
12800 positions has 75 pad rows after every 3125 real rows (pos = o +
(o//3125)*75). Table half h = concat over cores of shard[h*6400:(h+1)*6400];
chunks 0,1 in half 0; 2,3 in half 1. Zero rows sit at src_rel 3125..3199 of
every chunk (zero_rel = 3125).
"""
import numpy as np
import concourse.bass as bass
import concourse.bacc as bacc
import concourse.tile as tile
import concourse.mybir as mybir

F32 = mybir.dt.float32
F16 = mybir.dt.float16
I16 = mybir.dt.int16
AF = mybir.ActivationFunctionType
ALU = mybir.AluOpType

BUCKW = 512
SENTINEL = 300.0
RMAX = 12


class Cfg:
    def __init__(self, n_nodes, in_dim, hid, n_cores=8, buckw=BUCKW):
        self.n_nodes = n_nodes
        self.in_dim = in_dim
        self.hid = hid
        self.n_cores = n_cores
        self.buckw = buckw
        real = -(-n_nodes // n_cores)
        real = -(-real // 4) * 4
        self.real_per_core = real            # 12500
        self.real_quarter = real // 4        # 3125
        shard = -(-(real + 4) // buckw) * buckw
        self.shard = shard                   # 12800
        self.pad_q = (shard - real) // 4     # 75
        assert self.pad_q > 0 and (shard - real) % 4 == 0
        self.half = shard // 2               # 6400
        self.n_pad = shard * n_cores         # 102400
        self.blocks = shard // 128           # 100
        self.n_sb = shard // buckw           # 25
        self.chunk_rows = self.n_pad // 4    # 25600
        self.zero_rel = self.real_quarter    # 3125
        assert self.chunk_rows <= 32767
        assert self.half * self.n_cores == 2 * self.chunk_rows
        assert self.half % 128 == 0 and self.shard % buckw == 0
        assert self.half % (self.real_quarter + self.pad_q) == 0


def _pos_of(o, cfg):
    return o + (o // cfg.real_quarter) * cfg.pad_q


def preprocess(edge_index, cfg: Cfg):
    n = cfg.n_nodes
    real = cfg.real_per_core
    row = np.asarray(edge_index[0], dtype=np.int64)
    col = np.asarray(edge_index[1], dtype=np.int64)
    deg = (np.bincount(col, minlength=n) + 1).astype(np.float32)
    dinv = (1.0 / np.sqrt(deg)).astype(np.float32)
    # dinv in position space
    nodes = np.arange(n, dtype=np.int64)
    ncore = nodes // real
    npos = _pos_of(nodes % real, cfg)
    dinv_pos = np.zeros(cfg.n_pad, dtype=np.float32)
    dinv_pos[ncore * cfg.shard + npos] = dinv

    # src side
    score = row // real
    spos = _pos_of(row % real, cfg)
    h_s = spos // cfg.half
    rih = score * cfg.half + (spos % cfg.half)
    chunk = h_s * 2 + rih // cfg.chunk_rows
    src_rel = (rih % cfg.chunk_rows).astype(np.int16)
    # dest side
    dcore = col // real
    dpos = _pos_of(col % real, cfg)
    s_b = dpos // cfg.buckw
    j_b = (dpos % cfg.buckw) // 128
    dest_local = (dpos % 128).astype(np.int64)

    nsg = cfg.n_cores * cfg.n_sb * 4 * 4
    segid = (((dcore * cfg.n_sb + s_b) * 4 + chunk) * 4 + j_b).astype(np.int64)
    # sort by (segid, dest) and rank within each (segid, dest) run
    key = segid * 128 + dest_local
    order = np.argsort(key, kind="stable")
    key_s = key[order]
    seg_s = segid[order]
    dest_s = dest_local[order]
    srel_s = src_rel[order]
    # rank within (segid,dest)
    isnew = np.ones(len(key_s), dtype=bool)
    isnew[1:] = key_s[1:] != key_s[:-1]
    grp_start = np.maximum.accumulate(np.where(isnew, np.arange(len(key_s)), 0))
    rank = np.arange(len(key_s)) - grp_start
    # multiplicity table [ncores, n_sb, 4, 4, 128]
    mults = np.bincount(key, minlength=nsg * 128).reshape(
        cfg.n_cores, cfg.n_sb, 4, 4, 128)

    # choose R (identity rounds) per (s, ch, j): minimize tokens, then DVE ops
    best_R = np.zeros((cfg.n_sb, 4, 4), dtype=np.int64)
    best_C = np.zeros((cfg.n_sb, 4, 4), dtype=np.int64)
    costs = []
    for R in range(RMAX + 1):
        tail = np.maximum(mults - R, 0).sum(axis=-1)     # [cores, s, ch, j]
        C = -(-tail.max(axis=0) // 128)                  # [s, ch, j]
        costs.append((R + C, C))
    tok = np.stack([c[0] for c in costs])                # [R+1, s, ch, j]
    Cs = np.stack([c[1] for c in costs])
    # prefer fewer tokens; tie -> fewer classic groups (larger R)
    sel = np.lexsort((Cs.reshape(RMAX + 1, -1), tok.reshape(RMAX + 1, -1)),
                     axis=0)[0].reshape(cfg.n_sb, 4, 4)
    best_R = sel
    best_C = np.take_along_axis(Cs, sel[None], axis=0)[0]

    seg_tokens = (best_R + best_C) * 128                 # [s, ch, j]
    n_idx_tot = int(seg_tokens.sum())
    n_classic = int(best_C.sum())

    # per-(segid) slices of the sorted edge list
    seg_counts = np.bincount(seg_s, minlength=nsg)
    seg_starts = np.zeros(nsg + 1, dtype=np.int64)
    np.cumsum(seg_counts, out=seg_starts[1:])

    # spread pad tokens across all zero rows (same-row gathers bank-conflict)
    rq, pq = cfg.real_quarter, cfg.pad_q
    qspan = rq + pq
    nq_in_chunk = cfg.chunk_rows // qspan
    zero_rels = (np.arange(nq_in_chunk)[:, None] * qspan + rq
                 + np.arange(pq)[None, :]).ravel().astype(np.int16)
    per_core = []
    for c in range(cfg.n_cores):
        idx_flat = zero_rels[np.arange(n_idx_tot) % len(zero_rels)]
        dest_flat = np.full(n_classic * 128, SENTINEL, dtype=np.float32)
        off = 0
        coff = 0
        for s in range(cfg.n_sb):
            for ch in range(4):
                for j in range(4):
                    R = int(best_R[s, ch, j])
                    C = int(best_C[s, ch, j])
                    k = ((c * cfg.n_sb + s) * 4 + ch) * 4 + j
                    a, e = seg_starts[k], seg_starts[k + 1]
                    dl = dest_s[a:e]
                    rk = rank[a:e]
                    sr = srel_s[a:e]
                    ident_m = rk < R
                    idx_flat[off + rk[ident_m] * 128 + dl[ident_m]] = \
                        sr[ident_m]
                    tail_m = ~ident_m
                    ntail = int(tail_m.sum())
                    if ntail:
                        tpos = off + R * 128 + np.arange(ntail)
                        idx_flat[tpos] = sr[tail_m]
                        dest_flat[coff + np.arange(ntail)] = dl[tail_m]
                    off += (R + C) * 128
                    coff += C * 128
        assert off == n_idx_tot and coff == n_classic * 128
        wrapped = np.tile(idx_flat.reshape(n_idx_tot // 16, 16).T, (8, 1))
        if n_classic == 0:
            dest_flat = np.full(128, SENTINEL, dtype=np.float32)
        dest_w = dest_flat.reshape(max(n_classic, 1), 128).T.copy()
        dinvcol = dinv_pos[c * cfg.shard:(c + 1) * cfg.shard].reshape(
            cfg.blocks, 128).T.copy()
        dinvdest = np.tile(dinv_pos[c * cfg.shard:(c + 1) * cfg.shard],
                           (128, 1)).astype(np.float16)
        per_core.append(dict(idx=wrapped, dest=dest_w, dinvcol=dinvcol,
                             dinvdest=dinvdest))
    meta = dict(best_R=best_R, best_C=best_C, n_idx_tot=n_idx_tot,
                n_classic=n_classic)
    return per_core, meta, None


def build_kernel(cfg: Cfg, meta, sb_sizes=None, nqueues=4, repeat=1,
                 timing_loop=0, no_collectives=False):
    hid, ind = cfg.hid, cfg.in_dim
    best_R = np.asarray(meta["best_R"])
    best_C = np.asarray(meta["best_C"])
    n_idx_tot = int(meta["n_idx_tot"])
    n_classic = int(meta["n_classic"])
    kparts = ind // 128
    nc = bacc.Bacc("TRN2", target_bir_lowering=False, debug=False,
                   num_devices=cfg.n_cores, num_swdge_queues=nqueues)

    x_t = nc.dram_tensor("x_t", [ind, cfg.shard], F16, kind="ExternalInput")
    w1 = nc.dram_tensor("w1", [ind, hid], F16, kind="ExternalInput")
    w2 = nc.dram_tensor("w2", [hid, hid], F16, kind="ExternalInput")
    b1 = nc.dram_tensor("b1", [hid, 1], F32, kind="ExternalInput")
    b2 = nc.dram_tensor("b2", [hid, 1], F32, kind="ExternalInput")
    iota_in = nc.dram_tensor("iota", [128, 128], F16, kind="ExternalInput")
    ident_in = nc.dram_tensor("ident", [128, 128], F16, kind="ExternalInput")
    dinvcol_in = nc.dram_tensor("dinvcol", [128, cfg.blocks], F32,
                                kind="ExternalInput")
    dinvdest_in = nc.dram_tensor("dinvdest", [128, cfg.shard], F16,
                                 kind="ExternalInput")
    idx_in = nc.dram_tensor("idx", [128, n_idx_tot // 16], I16,
                            kind="ExternalInput")
    dest_in = nc.dram_tensor("dest", [128, max(n_classic, 1)], F32,
                             kind="ExternalInput")
    out = nc.dram_tensor("out", [hid, cfg.shard], F16, kind="ExternalOutput")

    rg = [list(range(cfg.n_cores))]

    # idx layout: s-major, ch, j; per segment: R ident groups then C classic
    seg_off = {}
    classic_off = {}
    _o = 0
    _co = 0
    for s in range(cfg.n_sb):
        for ch in range(4):
            for j in range(4):
                seg_off[(s, ch, j)] = _o
                classic_off[(s, ch, j)] = _co
                _o += int(best_R[s, ch, j] + best_C[s, ch, j]) * 128
                _co += int(best_C[s, ch, j])
    assert _o == n_idx_tot and _co == n_classic
    buck_len = {(s, ch): int((best_R[s, ch] + best_C[s, ch]).sum()) * 128
                for s in range(cfg.n_sb) for ch in range(4)}
    gmax = max(buck_len.values())
    half_rows = cfg.half
    hblocks = half_rows // 128
    bpb = cfg.buckw // 128  # 128-blocks per bucket (4)

    with tile.TileContext(nc) as tc:
        with (
            tc.tile_pool(name="dram", bufs=1, space="DRAM") as dram,
            tc.tile_pool(name="const", bufs=1) as cpool,
            tc.tile_pool(name="xin", bufs=4) as xpool,
            tc.tile_pool(name="mmps", bufs=2, space="PSUM") as mmps,
            tc.tile_pool(name="aggps", bufs=6, space="PSUM") as aggps,
            tc.tile_pool(name="gat", bufs=12) as gatpool,
            tc.tile_pool(name="sgen", bufs=12) as spool,
            tc.tile_pool(name="hst", bufs=4) as hpool,
        ):
            shard1h = [dram.tile([half_rows, hid], F16, tag=f"s1h{h}",
                                 name=f"s1h{h}") for h in range(2)]
            shard2h = [dram.tile([half_rows, hid], F16, tag=f"s2h{h}",
                                 name=f"s2h{h}") for h in range(2)]
            table1h = [dram.tile([2 * cfg.chunk_rows, hid], F16,
                                 tag=f"t1h{h}", name=f"t1h{h}")
                       for h in range(2)]
            table2h = [dram.tile([2 * cfg.chunk_rows, hid], F16,
                                 tag=f"t2h{h}", name=f"t2h{h}")
                       for h in range(2)]

            w1_t = cpool.tile([128, kparts, hid], F16, tag="w1")
            nc.sync.dma_start(w1_t[:], w1[:].rearrange("(k p) h -> p k h", p=128))
            dinvcol_t = cpool.tile([128, cfg.blocks], F32, tag="dinvcol")
            nc.sync.dma_start(dinvcol_t[:], dinvcol_in[:])

            iota_t = cpool.tile([128, 128], F16, tag="iota")
            nc.sync.dma_start(iota_t[:], iota_in[:])
            ident_t = cpool.tile([128, 128], F16, tag="ident")
            nc.sync.dma_start(ident_t[:], ident_in[:])
            w2_t = cpool.tile([128, hid], F16, tag="w2")
            nc.sync.dma_start(w2_t[:], w2[:])
            b1_t = cpool.tile([128, 1], F32, tag="b1")
            nc.sync.dma_start(b1_t[:], b1[:])
            b2_t = cpool.tile([128, 1], F32, tag="b2")
            nc.sync.dma_start(b2_t[:], b2[:])
            dinvdest_t = cpool.tile([128, cfg.shard], F16, tag="dinvdest")
            idx_t = cpool.tile([128, n_idx_tot // 16], I16, tag="idx")
            dest_t = cpool.tile([128, max(n_classic, 1)], F32, tag="dest")
            shard1_res = cpool.tile([128, cfg.blocks, hid], F16, tag="sh1")
            shard2_res = cpool.tile([128, cfg.blocks, hid], F16, tag="sh2")
            for _i in range(12):
                gz = gatpool.tile([128, gmax // 128, hid], F16, tag="gt")
                nc.vector.memset(gz[:], 0.0)

            if timing_loop:
                nc.sync.dma_start(dinvdest_t[:], dinvdest_in[:])
                nc.sync.dma_start(idx_t[:], idx_in[:])
                nc.sync.dma_start(dest_t[:], dest_in[:])
                if not no_collectives:
                    for h in range(2):
                        nc.gpsimd.collective_compute(
                            "AllGather", ALU.bypass, replica_groups=rg,
                            ins=[shard1h[h].opt()], outs=[table1h[h].opt()])
                    for h in range(2):
                        nc.gpsimd.collective_compute(
                            "AllGather", ALU.bypass, replica_groups=rg,
                            ins=[shard2h[h].opt()], outs=[table2h[h].opt()])
                loop_cm = tc.For_i(0, timing_loop, 1)
                loop_cm.__enter__()

            for _rep in range(repeat):
                # ---- prologue: y1 = dinv * (x @ W1) ----
                for n in range(cfg.blocks):
                    xt = xpool.tile([128, kparts, 128], F16)
                    nc.sync.dma_start(
                        xt[:], x_t[:, n * 128:(n + 1) * 128].rearrange(
                            "(k p) d -> p k d", p=128))
                    ps = mmps.tile([128, hid], F32, tag="mm")
                    for k in range(kparts):
                        nc.tensor.matmul(ps[:], xt[:, k, :], w1_t[:, k, :],
                                         start=(k == 0), stop=(k == kparts - 1))
                    nc.scalar.activation(shard1_res[:, n, :], ps[:], AF.Copy,
                                         scale=dinvcol_t[:, n:n + 1])
                    hh = n // hblocks
                    r0 = (n % hblocks) * 128
                    nc.sync.dma_start(shard1h[hh][r0:r0 + 128, :],
                                      shard1_res[:, n, :])
                    if n == 0 and _rep == 0 and not timing_loop:
                        # metadata loads overlap the prologue compute
                        nc.sync.dma_start(dinvdest_t[:], dinvdest_in[:])
                        nc.sync.dma_start(idx_t[:], idx_in[:])
                        nc.sync.dma_start(dest_t[:], dest_in[:])
                    if (n == hblocks - 1 and not timing_loop
                            and not no_collectives):
                        nc.gpsimd.collective_compute(
                            "AllGather", ALU.bypass, replica_groups=rg,
                            ins=[shard1h[0].opt()], outs=[table1h[0].opt()])

                if not timing_loop and not no_collectives:
                    nc.gpsimd.collective_compute(
                        "AllGather", ALU.bypass, replica_groups=rg,
                        ins=[shard1h[1].opt()], outs=[table1h[1].opt()])
                elif no_collectives and cfg.n_cores == 1:
                    for h in range(2):
                        nc.sync.dma_start(table1h[h][:cfg.half, :],
                                          shard1h[h][:, :])

                for layer in (1, 2):
                    tableh = table1h if layer == 1 else table2h
                    shard_res = shard1_res if layer == 1 else shard2_res
                    bias_t = b1_t if layer == 1 else b2_t
                    for s in range(cfg.n_sb):
                        gts = {}
                        for ch in range(4):
                            o = seg_off[(s, ch, 0)]
                            L = buck_len[(s, ch)]
                            if L == 0:
                                gts[ch] = (None, 0, 0)
                                continue
                            gt = gatpool.tile([128, L // 128, hid], F16,
                                              tag="gt")
                            src = tableh[ch // 2]
                            c0 = (ch % 2) * cfg.chunk_rows
                            nc.gpsimd.dma_gather(
                                gt[:], src[c0:c0 + cfg.chunk_rows, :],
                                idx_t[:, o // 16:(o + L) // 16], L, L, hid,
                                single_packet=False, queue_num=ch % nqueues)
                            gts[ch] = (gt, o, L)
                        # per-128-block independent accumulators
                        aggs = []
                        for j in range(bpb):
                            agg = aggps.tile([128, 128], F32, tag="agg")
                            n_mm_j = int((best_R[s, :, j]
                                          + best_C[s, :, j]).sum())
                            nb = s * bpb + j
                            nc.tensor.matmul(agg[:], shard_res[:, nb, :],
                                             ident_t[:], start=True,
                                             stop=(n_mm_j == 0))
                            aggs.append([agg, n_mm_j, 0])
                        for ch in range(4):
                            (gt, o, L) = gts[ch]
                            if L == 0:
                                continue
                            for j in range(bpb):
                                R = int(best_R[s, ch, j])
                                C = int(best_C[s, ch, j])
                                base = (seg_off[(s, ch, j)] - o) // 128
                                agg_e = aggs[j]
                                for r in range(R):
                                    agg_e[2] += 1
                                    nc.tensor.matmul(
                                        agg_e[0][:], gt[:, base + r, :],
                                        ident_t[:], start=False,
                                        stop=(agg_e[2] == agg_e[1]))
                                for q in range(C):
                                    G = classic_off[(s, ch, j)] + q
                                    s_t = spool.tile([128, 128], F16, tag="s")
                                    nc.vector.tensor_scalar(
                                        s_t[:], iota_t[:], dest_t[:, G:G + 1],
                                        None, ALU.is_equal)
                                    agg_e[2] += 1
                                    nc.tensor.matmul(
                                        agg_e[0][:], gt[:, base + R + q, :],
                                        s_t[:], start=False,
                                        stop=(agg_e[2] == agg_e[1]))
                        hd = hpool.tile([128, cfg.buckw], F16, tag="hd")
                        for j in range(bpb):
                            c0 = s * cfg.buckw + j * 128
                            nc.vector.tensor_tensor(
                                hd[:, j * 128:(j + 1) * 128], aggs[j][0][:],
                                dinvdest_t[:, c0:c0 + 128], ALU.mult)
                        h_t = hpool.tile([128, cfg.buckw], F16, tag="ht")
                        nc.scalar.activation(h_t[:], hd[:], AF.Tanh,
                                             bias=bias_t[:])
                        if layer == 1:
                            for jj in range(bpb):
                                nb = s * bpb + jj
                                ps2 = mmps.tile([128, hid], F32, tag="mm")
                                nc.tensor.matmul(
                                    ps2[:], h_t[:, jj * 128:(jj + 1) * 128],
                                    w2_t[:], start=True, stop=True)
                                nc.scalar.activation(
                                    shard2_res[:, nb, :], ps2[:], AF.Copy,
                                    scale=dinvcol_t[:, nb:nb + 1])
                                hh = nb // hblocks
                                r0 = (nb % hblocks) * 128
                                nc.sync.dma_start(shard2h[hh][r0:r0 + 128, :],
                                                  shard2_res[:, nb, :])
                        else:
                            nc.sync.dma_start(
                                out[:, s * cfg.buckw:(s + 1) * cfg.buckw],
                                h_t[:])
                        if (layer == 1 and s * bpb + bpb - 1 >= hblocks - 1
                                and (s - 1) * bpb + bpb - 1 < hblocks - 1
                                and not timing_loop and not no_collectives):
                            nc.gpsimd.collective_compute(
                                "AllGather", ALU.bypass, replica_groups=rg,
                                ins=[shard2h[0].opt()], outs=[table2h[0].opt()])
                    if layer == 1 and not timing_loop and not no_collectives:
                        nc.gpsimd.collective_compute(
                            "AllGather", ALU.bypass, replica_groups=rg,
                            ins=[shard2h[1].opt()], outs=[table2h[1].opt()])
                    elif layer == 1 and no_collectives and cfg.n_cores == 1:
                        for h in range(2):
                            nc.sync.dma_start(table2h[h][:cfg.half, :],
                                              shard2h[h][:, :])
            if timing_loop:
                loop_cm.__exit__(None, None, None)
    nc.compile()
    return nc


def make_runner(nc, n_cores):
    """Build a cached jitted executor for nc (avoids per-call re-jit + NEFF
    reload). Returns run(in_maps) -> list of per-core {name: np.ndarray}."""
    import jax
    import numpy as np
    from jax.sharding import Mesh, PartitionSpec
    from jax.experimental.shard_map import shard_map
    from concourse import bass2jax
    from concourse.bass2jax import _bass_exec_p, partition_id_tensor

    bass2jax.install_neuronx_cc_hook()
    in_names, out_names, out_avals, zero_outs = [], [], [], []
    pname = nc.partition_id_tensor.name if nc.partition_id_tensor else None
    for alloc in nc.m.functions[0].allocations:
        if not isinstance(alloc, mybir.MemoryLocationSet):
            continue
        name = alloc.memorylocations[0].name
        if alloc.kind == "ExternalInput":
            if name != pname:
                in_names.append(name)
        elif alloc.kind == "ExternalOutput":
            shape = tuple(alloc.tensor_shape)
            dtype = mybir.dt.np(alloc.dtype)
            out_names.append(name)
            out_avals.append(jax.core.ShapedArray(shape, dtype))
            zero_outs.append(np.zeros(shape, dtype))
    n_params = len(in_names)
    all_in = list(in_names) + list(out_names)
    if pname is not None:
        all_in.append(pname)

    def _body(*args):
        operands = list(args)
        if pname is not None:
            operands.append(partition_id_tensor())
        outs = _bass_exec_p.bind(
            *operands,
            out_avals=tuple(out_avals),
            in_names=tuple(all_in),
            out_names=tuple(out_names),
            lowering_input_output_aliases=(),
            sim_require_finite=True,
            sim_require_nnan=True,
            nc=nc,
        )
        return tuple(outs)

    donate = tuple(range(n_params, n_params + len(out_avals)))
    if n_cores == 1:
        fn = jax.jit(_body, donate_argnums=donate, keep_unused=True)
        import jax.numpy as jnp
        zmk1 = jax.jit(lambda: tuple(jnp.zeros(z.shape, z.dtype)
                                     for z in zero_outs))
        state1 = {}

        def run(in_maps, fetch=True):
            if state1.get("key") is not id(in_maps):
                state1["dev_in"] = [jax.device_put(np.asarray(in_maps[0][n]))
                                    for n in in_names]
                state1["key"] = id(in_maps)
            outs = fn(*state1["dev_in"], *zmk1())
            if not fetch:
                jax.block_until_ready(outs)
                return None
            return [{n: np.asarray(outs[i]) for i, n in enumerate(out_names)}]
        return run

    devices = jax.devices()[:n_cores]
    mesh = Mesh(np.asarray(devices), ("core",))
    in_specs = (PartitionSpec("core"),) * (n_params + len(out_avals))
    out_specs = (PartitionSpec("core"),) * len(out_names)
    fn = jax.jit(
        shard_map(_body, mesh=mesh, in_specs=in_specs, out_specs=out_specs,
                  check_rep=False),
        donate_argnums=donate, keep_unused=True)

    from jax.sharding import NamedSharding
    shard_spec = NamedSharding(mesh, PartitionSpec("core"))
    zero_shapes = [(n_cores * z.shape[0], *z.shape[1:]) for z in zero_outs]
    zero_dtypes = [z.dtype for z in zero_outs]
    import jax.numpy as jnp
    zmk = jax.jit(
        lambda: tuple(jnp.zeros(s, d) for s, d in zip(zero_shapes, zero_dtypes)),
        out_shardings=tuple(shard_spec for _ in zero_shapes))
    state = {}

    def run(in_maps, fetch=True):
        # cache device-resident inputs keyed on the in_maps list identity:
        # repeated timing calls pass the same object and skip the upload.
        if state.get("key") is not id(in_maps):
            concat_in = [
                np.concatenate([np.asarray(in_maps[c][n])
                                for c in range(n_cores)], axis=0)
                for n in in_names]
            state["dev_in"] = [jax.device_put(a, shard_spec) for a in concat_in]
            state["key"] = id(in_maps)
        outs = fn(*state["dev_in"], *zmk())
        if not fetch:
            jax.block_until_ready(outs)
            return None
        return [
            {n: np.asarray(outs[i]).reshape(n_cores, *out_avals[i].shape)[c]
             for i, n in enumerate(out_names)}
            for c in range(n_cores)]
    return run




def make_in_maps(x, w1, b1, w2, b2, cfg: Cfg, per_core_meta):
    iota = np.tile(np.arange(128, dtype=np.float16), (128, 1))
    ident = np.eye(128, dtype=np.float16)
    xpad = np.zeros((cfg.n_pad, cfg.in_dim), dtype=np.float16)
    x16 = np.asarray(x, np.float16)
    real = cfg.real_per_core
    nodes = np.arange(cfg.n_nodes, dtype=np.int64)
    gpos = (nodes // real) * cfg.shard + _pos_of(nodes % real, cfg)
    xpad[gpos] = x16
    maps = []
    for c in range(cfg.n_cores):
        xs = xpad[c * cfg.shard:(c + 1) * cfg.shard]
        maps.append({
            "x_t": np.ascontiguousarray(xs.T),
            "w1": np.asarray(w1, np.float16),
            "w2": np.asarray(w2, np.float16),
            "b1": np.asarray(b1, np.float32).reshape(-1, 1),
            "b2": np.asarray(b2, np.float32).reshape(-1, 1),
            "iota": iota,
            "ident": ident,
            "dinvcol": per_core_meta[c]["dinvcol"],
            "dinvdest": per_core_meta[c]["dinvdest"],
            "idx": per_core_meta[c]["idx"],
            "dest": per_core_meta[c]["dest"],
        })
    return maps


def assemble_output(results, cfg: Cfg):
    outs = [np.asarray(r["out"]).T.astype(np.float32) for r in results]
    full = np.concatenate(outs, axis=0)        # [n_pad, hid] in pos space
    real = cfg.real_per_core
    nodes = np.arange(cfg.n_nodes, dtype=np.int64)
    gpos = (nodes // real) * cfg.shard + _pos_of(nodes % real, cfg)
    return full[gpos]


_CACHE = {}

N_NODES = 100000
IN_DIM = 256
HID_DIM = 128
N_CORES = 8
CFG_KW = dict(buckw=BUCKW)


def kernel(x, edge_index, W1, b1, W2, b2):
    x = np.asarray(x, dtype=np.float32)
    edge_index = np.asarray(edge_index)
    cfg = Cfg(N_NODES, IN_DIM, HID_DIM, n_cores=N_CORES, **CFG_KW)
    per_core, meta, sb_sizes = preprocess(edge_index, cfg)
    key = (bytes(np.asarray(meta["best_R"])), bytes(np.asarray(meta["best_C"])))
    if key not in _CACHE:
        nc = build_kernel(cfg, meta, sb_sizes, nqueues=4)
        run = make_runner(nc, N_CORES)
        _CACHE[key] = run
    run = _CACHE[key]
    maps = make_in_maps(x, W1, b1, W2, b2, cfg, per_core)
    results = run(maps)
    return np.ascontiguousarray(assemble_output(results, cfg).astype(np.float32))
